# revision 16
# baseline (speedup 1.0000x reference)
"""MoE gate (softmax + top-2 + seq-aux-loss) Trainium2 Bass kernel.

Problem: hidden_states [4, 4096, 2048] f32, weight [64, 2048] f32, top_k=2.
  logits = flat @ W.T          [16384, 64]
  scores = softmax(logits)
  topk_weight, topk_idx = top_k(scores, 2); weights renormalized
  aux_loss = seq-level load-balancing loss (scalar)

Sharding: token dim across 8 cores (2048 tokens/core; each core's tokens lie
within a single batch). W.T replicated. Aux-loss partial sums (per-expert
selection counts + score sums) are computed per core and combined on host.
"""

import numpy as np

# ---- problem constants (hardcoded per harness contract) ----
B, S, H, E = 4, 4096, 2048, 64
TOPK = 2
NCORES = 8
T = B * S                 # 16384 tokens
TPC = T // NCORES         # 2048 tokens per core
P = 128                   # partitions
G = TPC // P              # 16 token groups of 128 per core
HC = H // P               # 16 contraction chunks of 128
GH = G // 2               # groups per half-megatile (8)
ALPHA = 0.1
NEG_BIG = -1e30

_CACHE = {}


def _build_module(split=True):
    import concourse.bass as bass
    import concourse.mybir as mybir
    import concourse.tile as tile
    from concourse.tile import add_dep_helper
    from contextlib import ExitStack

    f32 = mybir.dt.float32
    i32 = mybir.dt.int32
    Alu = mybir.AluOpType
    Act = mybir.ActivationFunctionType
    X = mybir.AxisListType.X

    nc = bass.Bass("TRN2", target_bir_lowering=False, debug=False,
                   num_devices=NCORES)

    x_ap = nc.dram_tensor("x", [TPC, H], f32, kind="ExternalInput").ap()
    wt_ap = nc.dram_tensor("wt", [H, E], f32, kind="ExternalInput").ap()
    idx_out = nc.dram_tensor("idx_out", [TPC, TOPK], i32,
                             kind="ExternalOutput").ap()
    w_out = nc.dram_tensor("w_out", [TPC, TOPK], f32,
                           kind="ExternalOutput").ap()
    # per-core aux partials: [half, {counts(64) | score_sums(64)}]
    aux_out = nc.dram_tensor("aux_out", [1, 2 * P], f32,
                             kind="ExternalOutput").ap()

    with tile.TileContext(nc) as tc, ExitStack() as ctx:
        singles = ctx.enter_context(tc.tile_pool(name="singles", bufs=1))
        xpool = ctx.enter_context(tc.tile_pool(name="xp", bufs=3))
        ftpool = ctx.enter_context(tc.tile_pool(name="ftp", bufs=4))
        spool = ctx.enter_context(tc.tile_pool(name="sp", bufs=1))
        hpool = ctx.enter_context(tc.tile_pool(name="hp", bufs=2))
        tpsum = ctx.enter_context(tc.tile_pool(name="tps", bufs=2,
                                               space="PSUM"))
        lpsum = ctx.enter_context(tc.tile_pool(name="lps", bufs=1,
                                               space="PSUM"))
        apsum = ctx.enter_context(tc.tile_pool(name="aps", bufs=1,
                                               space="PSUM"))

        # --- constants ---
        identity = singles.tile([P, P], f32)
        nc.gpsimd.memset(identity, 0.0)
        id_inst = nc.gpsimd.affine_select(
            out=identity, in_=identity, compare_op=Alu.not_equal, fill=1.0,
            base=0, pattern=[[-1, P]], channel_multiplier=1)
        wt_sb = singles.tile([P, HC, E], f32)   # wt_sb[p, c, e] = W.T[c*128+p, e]
        wt_dma = nc.sync.dma_start(wt_sb, wt_ap.rearrange("(c p) e -> p c e",
                                                          p=P))
        iota_i = singles.tile([P, E], i32)
        nc.gpsimd.iota(iota_i, pattern=[[1, E]], base=0, channel_multiplier=0)
        iota_f = singles.tile([P, E], f32)
        nc.vector.tensor_copy(iota_f, iota_i)
        ones = singles.tile([P, 1], f32)
        nc.vector.memset(ones, 1.0)

        _ = id_inst, add_dep_helper, wt_dma

        # --- output staging ---
        idx_sb = spool.tile([P, G, TOPK], i32)
        w_sb = spool.tile([P, G, TOPK], f32)
        aux_cat = spool.tile([P, 2, 2 * E], f32)  # [p, half, counts|ssum]

        # logits PSUM accumulators, one bank per half (8 groups x 64)
        logits_ps = [lpsum.tile([P, GH * E], f32, tag=f"lp{h}",
                                name=f"lp{h}")
                     for h in range(2)]

        def mm_phase(g):
            """transpose x tile g and accumulate logits for its 128 tokens."""
            x_tile = xpool.tile([P, H], f32, tag="x")
            nc.sync.dma_start(x_tile, x_ap[g * P:(g + 1) * P, :])
            lsl = logits_ps[g // GH][:, (g % GH) * E:(g % GH) * E + E]
            for q in range(HC // 4):
                ftp = tpsum.tile([P, 4 * P], f32, tag="ftp")
                for j in range(4):
                    h = q * 4 + j
                    nc.tensor.transpose(ftp[:, j * P:(j + 1) * P],
                                        x_tile[:, h * P:(h + 1) * P],
                                        identity)
                fts = ftpool.tile([P, 4 * P], f32, tag="fts")
                nc.scalar.copy(fts, ftp)
                for j in range(4):
                    h = q * 4 + j
                    nc.tensor.matmul(lsl, fts[:, j * P:(j + 1) * P],
                                     wt_sb[:, h, :],
                                     start=(h == 0), stop=(h == HC - 1))

        def routing_phase(hh):
            """softmax/top2/aux for half hh (8 token groups, [128, 8, 64])."""
            lp3 = logits_ps[hh].rearrange("p (g e) -> p g e", e=E)
            sh = [P, GH, E]
            logits = hpool.tile(sh, f32, tag="logits")
            nc.scalar.copy(logits, lp3)
            exp = hpool.tile(sh, f32, tag="exp")
            nc.scalar.activation(exp, logits, Act.Exp)

            m1 = hpool.tile([P, GH], f32, tag="m1")
            nc.vector.tensor_reduce(m1, logits, axis=X, op=Alu.max)
            mask1 = hpool.tile(sh, f32, tag="mask1")
            nc.vector.tensor_tensor(mask1, logits, m1.to_broadcast(sh),
                                    op=Alu.is_ge)
            masked = hpool.tile(sh, f32, tag="masked")
            nc.vector.scalar_tensor_tensor(masked, mask1, NEG_BIG, logits,
                                           op0=Alu.mult, op1=Alu.add)
            m2 = hpool.tile([P, GH], f32, tag="m2")
            nc.vector.tensor_reduce(m2, masked, axis=X, op=Alu.max)
            mask12 = hpool.tile(sh, f32, tag="mask12")
            nc.vector.tensor_tensor(mask12, logits, m2.to_broadcast(sh),
                                    op=Alu.is_ge)
            # per-expert counts for this half: sum over groups
            nc.vector.tensor_reduce(aux_cat[:, hh, 0:E],
                                    mask12.rearrange("p g e -> p e g"),
                                    axis=X, op=Alu.add)
            # softmax score sums per expert
            z = hpool.tile([P, GH], f32, tag="z")
            nc.vector.tensor_reduce(z, exp, axis=X, op=Alu.add)
            rz = hpool.tile([P, GH], f32, tag="rz")
            nc.vector.reciprocal(rz, z)
            scaled = hpool.tile(sh, f32, tag="scaled")
            nc.vector.tensor_tensor(scaled, exp, rz.to_broadcast(sh),
                                    op=Alu.mult)
            nc.vector.tensor_reduce(aux_cat[:, hh, E:2 * E],
                                    scaled.rearrange("p g e -> p e g"),
                                    axis=X, op=Alu.add)
            # top-2 indices via iota dot
            iob = bass.AP(iota_f.tensor, iota_f.offset,
                          [iota_f.ap[0], [0, GH], iota_f.ap[1]])
            t1 = hpool.tile(sh, f32, tag="t1")
            nc.vector.tensor_tensor(t1, mask1, iob, op=Alu.mult)
            idx1f = hpool.tile([P, GH], f32, tag="idx1f")
            nc.vector.tensor_reduce(idx1f, t1, axis=X, op=Alu.add)
            t2 = hpool.tile(sh, f32, tag="t2")
            nc.vector.tensor_tensor(t2, mask12, iob, op=Alu.mult)
            s12 = hpool.tile([P, GH], f32, tag="s12")
            nc.vector.tensor_reduce(s12, t2, axis=X, op=Alu.add)
            idx2f = hpool.tile([P, GH], f32, tag="idx2f")
            nc.vector.tensor_sub(idx2f, s12, idx1f)
            # top-2 weights: e_i / (e1 + e2) with e_i = exp(m_i)
            e1 = hpool.tile([P, GH], f32, tag="e1")
            nc.scalar.activation(e1, m1, Act.Exp)
            e2 = hpool.tile([P, GH], f32, tag="e2")
            nc.scalar.activation(e2, m2, Act.Exp)
            es = hpool.tile([P, GH], f32, tag="es")
            nc.vector.tensor_add(es, e1, e2)
            rs = hpool.tile([P, GH], f32, tag="rs")
            nc.vector.reciprocal(rs, es)
            gsl = slice(hh * GH, (hh + 1) * GH)
            nc.vector.tensor_mul(w_sb[:, gsl, 0], e1, rs)
            nc.vector.tensor_mul(w_sb[:, gsl, 1], e2, rs)
            nc.vector.tensor_copy(idx_sb[:, gsl, 0], idx1f)
            nc.vector.tensor_copy(idx_sb[:, gsl, 1], idx2f)

        for g in range(GH):
            mm_phase(g)
        routing_phase(0)
        for g in range(GH, G):
            mm_phase(g)
        routing_phase(1)

        # partition-reduce aux partials: ones.T @ aux_cat -> [1, 256]
        aux_ps = apsum.tile([1, 4 * E], f32)
        nc.tensor.matmul(aux_ps, ones,
                         aux_cat.rearrange("p h c -> p (h c)"),
                         start=True, stop=True)
        aux_sb = spool.tile([1, 4 * E], f32)
        nc.scalar.copy(aux_sb, aux_ps)
        nc.sync.dma_start(aux_out, aux_sb)

        # outputs: token t = g*128 + p
        nc.sync.dma_start(idx_out.rearrange("(g p) k -> p g k", p=P), idx_sb)
        nc.sync.dma_start(w_out.rearrange("(g p) k -> p g k", p=P), w_sb)

    if split:
        _split_multiwaits(nc, mybir)
    return nc


def _split_multiwaits(nc, mybir, max_waits=1):
    """Some engine-level ISA structs (fp32 self-loading matmul S3_LW,
    PSEUDO_DMA_DIRECT2D, ...) can carry only one sync-wait command.
    Sequencers execute waits in program order, so moving excess waits onto
    an InstNoOp placed immediately before the instruction is semantically
    identical. Walk the scheduled program and split any over-limit waits."""
    seq_like = ("InstUnconditionalBranch", "InstRegisterMove")
    for f in nc.m.functions:
        for blk in f.blocks:
            insts = list(blk.instructions)
            out, changed = [], False
            for inst in insts:
                si = inst.sync_info
                if (si is not None and len(si.on_wait) > max_waits
                        and type(inst).__name__ not in seq_like
                        and getattr(inst, "engine", None) is not None):
                    waits = list(si.on_wait)
                    for wi, w in enumerate(waits[:-max_waits]):
                        nop = mybir.InstNoOp(
                            name=f"{inst.name}-wn{wi}",
                            sync_info=mybir.SyncInfo(
                                on_wait=[w], on_update=[]),
                            bass_nofuse=True,
                            engine=inst.engine,
                        )
                        out.append(nop)
                    inst.sync_info = mybir.SyncInfo(
                        on_wait=waits[-max_waits:],
                        on_update=list(si.on_update))
                    changed = True
                out.append(inst)
            if changed:
                try:
                    blk.instructions = out
                except Exception:
                    blk.instructions[:] = out


def _get_module():
    if "nc" not in _CACHE:
        _CACHE["nc"] = _build_module()
    return _CACHE["nc"]


def kernel(hidden_states, weight, top_k):
    assert int(top_k) == TOPK
    from concourse.bass_utils import run_bass_kernel_spmd

    nc = _get_module()

    flat = np.ascontiguousarray(
        np.asarray(hidden_states, dtype=np.float32).reshape(T, H))
    wt = np.ascontiguousarray(np.asarray(weight, dtype=np.float32).T)
    in_maps = [{"x": flat[c * TPC:(c + 1) * TPC], "wt": wt}
               for c in range(NCORES)]

    res = run_bass_kernel_spmd(nc, in_maps, core_ids=list(range(NCORES)))
    results = res.results

    topk_idx = np.concatenate([r["idx_out"] for r in results], axis=0)
    topk_weight = np.concatenate([r["w_out"] for r in results], axis=0)

    # aux partials: [core, 2 halves, {counts(64) | ssum(64)}]
    parts = np.stack([r["aux_out"][0].reshape(2, 2 * E) for r in results])
    counts = parts[:, :, :E].sum(axis=1)       # [8, 64]
    ssum = parts[:, :, E:].sum(axis=1)         # [8, 64]
    counts_b = counts.reshape(B, 2, E).sum(axis=1)   # [4, 64]
    ssum_b = ssum.reshape(B, 2, E).sum(axis=1)       # [4, 64]
    ce = counts_b * (E / (S * TOPK))
    mean_scores = ssum_b / S
    aux_loss = np.float32((ce * mean_scores).sum(axis=1).mean() * ALPHA)

    return topk_idx, topk_weight, aux_loss
